# revision 32
# baseline (speedup 1.0000x reference)
"""Causal GQA SDPA on 8 Trainium2 NeuronCores (Bass/Tile).

Problem: B=2, S=2048, NH=32 query heads, NKV=8 kv heads, D=128, f32 I/O,
causal additive mask. Sharding: tensor-parallel over query heads — core c
gets q heads [4c, 4c+4) for both batches, which map exactly onto kv head c
(GQA group size 4), so k/v need no replication across cores.

Per-core kernel (all compute in bf16, f32 PSUM accumulation):
  scores^T[k, q] = K^T(stationary) x Q^T(moving)  -> PSUM [128k, 512q]
  P^T = exp(scale * scores^T) via ScalarE          -> SBUF bf16
  causal: upper-triangle zeroed via gpsimd.affine_select on the diagonal
          128x128 block; fully-masked blocks are never computed.
  out[q, d+1] = P^T(stationary) x [V | 1](moving)  -> PSUM [128q, 129]
  column 128 accumulates the softmax denominator; divide via DVE
  reciprocal + per-partition tensor_scalar_mul, then DMA out f32.

No max-subtraction is needed: scores ~ N(0,1) after scaling, exp is far
from overflow, and exp(score - 1e9) underflows to exactly 0.0 in f32 just
like the reference's softmax(score + mask).
"""

import math
import numpy as np
import ml_dtypes

B = 2
S = 2048
NH = 32
NKV = 8
D = 128
NCORES = 8
HPC = NH // NCORES          # q heads per core = 4
QG = 4                      # q-groups of 512 per (b, h)
QBLK = 128                  # q rows per PSUM out tile
KT = 128                    # k rows per k-tile
NKT = S // KT               # 16 k-tiles
SCALE = 1.0 / math.sqrt(D)

# Schraudolph-style exp on the DVE: i16 = trunc_f32(A*s + B) bitcast to bf16
# approximates exp(s*SCALE) (rel err ~1.8% rms, calibrated C=-7 for the
# truncating f32->i16 convert). Used to offload part of the softmax exp from
# the Scalar engine (the bottleneck) onto the otherwise-idle Vector engine.
EXP_A = float(128.0 * math.log2(math.e) * SCALE)
EXP_B = float(127 * 128 - 7.0)

_CACHE = {}


def _split_waits(nc, max_waits=1):
    """The walrus build in this container rejects instructions carrying more
    than one sync-wait ("Too many sync wait commands"). Engine queues
    dispatch in order, so excess waits can ride on NOPs inserted just before
    the instruction on the same engine — semantically identical gating."""
    import concourse.mybir as mybir

    n = 0
    for fn in nc.m.functions:
        for bb in fn.blocks:
            new = []
            changed = False
            for ins in bb.instructions:
                si = ins.sync_info
                waits = list(si.on_wait) if si is not None and si.on_wait else []
                if len(waits) > max_waits:
                    for w in waits[:-max_waits]:
                        n += 1
                        nop = mybir.InstNoOp(
                            name=f"I-waitsplit-{n}", ins=[], outs=[]
                        )
                        nop.engine = ins.engine
                        nop.sync_info = mybir.SyncInfo(on_wait=[w], on_update=[])
                        new.append(nop)
                    ins.sync_info = mybir.SyncInfo(
                        on_wait=waits[-max_waits:], on_update=list(si.on_update)
                    )
                    changed = True
                new.append(ins)
            if changed:
                bb.instructions = new


def _build_nc():
    import concourse.bass as bass
    import concourse.mybir as mybir

    f32 = mybir.dt.float32
    bf16 = mybir.dt.bfloat16

    nc = bass.Bass()
    qT = nc.declare_dram_parameter("qT", [B, HPC, D, S], bf16, isOutput=False)
    kT = nc.declare_dram_parameter("kT", [B, D, S], bf16, isOutput=False)
    v = nc.declare_dram_parameter("v", [B, 4, KT, 4 * (D + 1)], bf16, isOutput=False)
    out = nc.declare_dram_parameter(
        "out", [B, HPC, S // QBLK, QBLK, D], f32, isOutput=True
    )

    from concourse.tile import TileContext

    with TileContext(nc) as tc:
        with (
            tc.tile_pool(name="kv", bufs=1) as kv_pool,
            tc.tile_pool(name="q", bufs=8) as q_pool,
            tc.tile_pool(name="pt", bufs=8) as pt_pool,
            tc.tile_pool(name="res", bufs=12) as res_pool,
            tc.tile_pool(name="st", bufs=3, space="PSUM") as st_pool,
            tc.tile_pool(name="acc", bufs=2, space="PSUM") as acc_pool,
        ):
            # Persistent K^T and V~ per batch, in chunk-granular tiles so the
            # first QK matmul waits on ~0.75MB of DMA, not ~3MB (Tile tracks
            # dependencies per tile).
            kt_sb = {}  # (b, ch) -> [D, 512] K^T cols for k-tiles 4ch..4ch+3
            v_sb = {}   # (b, ch) -> [KT, 4, D+1] V rows + ones col


            def load_kv_chunk(b, ch, what="kv", split=False):
                if "k" in what:
                    k_tile = kv_pool.tile(
                        [D, 512], bf16, tag=f"kt{b}c{ch}", name=f"ktile{b}{ch}"
                    )
                    if split:
                        # 4 sub-DMAs land on parallel queues so the first
                        # QK's 128-col slice arrives ~4x sooner at startup
                        for i in range(4):
                            nc.sync.dma_start(
                                k_tile[:, i * 128 : (i + 1) * 128],
                                kT[b][:, ch * 512 + i * 128 : ch * 512 + (i + 1) * 128],
                            )
                    else:
                        nc.sync.dma_start(
                            k_tile[:], kT[b][:, ch * 512 : (ch + 1) * 512]
                        )
                    kt_sb[(b, ch)] = k_tile
                if "v" in what:
                    # host layout [KT, 4*(D+1)]: per k-tile 129 cols, ones first
                    v_tile = kv_pool.tile(
                        [KT, 4, D + 1], bf16, tag=f"v{b}c{ch}", name=f"vtile{b}{ch}"
                    )
                    nc.sync.dma_start(v_tile[:], v[b, ch])
                    v_sb[(b, ch)] = v_tile

            def qk_lhsT(b, kt_i):
                return kt_sb[(b, kt_i // 4)][:, (kt_i % 4) * KT : (kt_i % 4 + 1) * KT]

            def pv_rhs(b, kt_i):
                return v_sb[(b, kt_i // 4)][:, kt_i % 4, :]

            # Global software pipeline: PV/exp consumers of pair p are
            # emitted two pairs behind its QK matmuls, so in PE program
            # order two more QK groups (plus older PVs) separate scores
            # production from probability consumption — enough slack
            # (~1.7us of PE work) to hide the ~1.2us exp latency.
            pending = []
            PIPE_DEPTH = 2

            def push_pending(fn):
                pending.append(fn)
                while len(pending) > PIPE_DEPTH:
                    pending.pop(0)()

            def flush_pending():
                while pending:
                    pending.pop(0)()

            bh_list = [(b, h) for b in range(B) for h in range(HPC)]
            q_sb_all = {}

            def load_q(b, h, split=False):
                q_sb_all[(b, h)] = {}
                for qg in range(QG):
                    q_tile = q_pool.tile(
                        [D, 512], bf16, tag=f"qg{qg}", name=f"qtile{qg}"
                    )
                    if split and qg == 0:
                        for i in range(4):
                            nc.sync.dma_start(
                                q_tile[:, i * 128 : (i + 1) * 128],
                                qT[b, h][:, i * 128 : (i + 1) * 128],
                            )
                    else:
                        nc.sync.dma_start(
                            q_tile[:], qT[b, h][:, qg * 512 : (qg + 1) * 512]
                        )
                    q_sb_all[(b, h)][qg] = q_tile
                    if h == 0:
                        # pair0 deps (q0,k0) first; v0 right after
                        load_kv_chunk(b, qg, "k", split=(split and qg == 0))
                        load_kv_chunk(b, qg, "v")

            for idx, (b, h) in enumerate(bh_list):
                    if idx == 0:
                        load_q(b, h, split=True)
                    q_sb = q_sb_all[(b, h)]
                    ip_counter = [0]

                    for qg in range(QG):
                        if qg == 2 and idx + 1 < len(bh_list):
                            # prefetch next head's inputs mid-compute so the
                            # h-boundary has no DMA-queue collision
                            load_q(*bh_list[idx + 1])
                        n_kt = 4 * qg + 4
                        # two q-block accumulators share one PSUM bank
                        # ([128, 2, 129] f32 = 1032B/partition) so all four
                        # fit in 2 banks, freeing space for st triple-buffering
                        acc_t = [
                            acc_pool.tile(
                                [QBLK, 2, D + 1], f32, tag="acc", name=f"acc{i}"
                            )
                            for i in range(2)
                        ]
                        out_ps = [acc_t[i // 2][:, i % 2, :] for i in range(4)]

                        def res_drain_bank(t, qg=qg, b=b, h=h, acc_t=acc_t):
                            # normalize + store the 2 q-blocks of acc bank t.
                            # Both its chains close with diag pair t, one pair
                            # before the qg ends for bank 0 — draining per
                            # bank unblocks the next qg's PV WAR early while
                            # PE only ever writes the OTHER bank (collision-
                            # safe). Reciprocal batched on the DVE; one
                            # multiply on ScalarE, one on the DVE (balance).
                            rec = res_pool.tile([QBLK, 2], f32, tag=f"rec{t}")
                            nc.vector.reciprocal(rec[:], acc_t[t][:, :, 0])
                            for half in range(2):
                                qb = 2 * t + half
                                osb = res_pool.tile([QBLK, D], f32, tag="osb")
                                acc_ap = acc_t[t][:, half, 1 : D + 1]
                                rec_ap = rec[:, half : half + 1]
                                if half == 1:
                                    nc.vector.tensor_scalar_mul(
                                        osb[:], acc_ap, rec_ap
                                    )
                                else:
                                    nc.scalar.activation(
                                        osb[:],
                                        acc_ap,
                                        mybir.ActivationFunctionType.Copy,
                                        bias=0.0,
                                        scale=rec_ap,
                                    )
                                nc.sync.dma_start(out[b, h, qg * 4 + qb], osb[:])

                        # k-tiles in pairs: one [128,1024] PSUM tile and one
                        # wide ACTIVATE (amortizes the 352-cycle overhead).
                        for ktp in range(n_kt // 2):
                            kt0 = 2 * ktp
                            st = st_pool.tile([KT, 1024], f32)
                            pt = pt_pool.tile([KT, 1024], bf16, tag="pt")
                            offs = []
                            for half in range(2):
                                kt_i = kt0 + half
                                j = kt_i - 4 * qg  # >= 0 on the diagonal band
                                q_off = max(0, j) * QBLK
                                offs.append(q_off)
                                nc.tensor.matmul(
                                    st[:, half * 512 + q_off : (half + 1) * 512],
                                    lhsT=qk_lhsT(b, kt_i),
                                    rhs=q_sb[qg][:, q_off:512],
                                    start=True,
                                    stop=True,
                                )

                            is_diag = kt0 + 1 >= 4 * qg
                            # pair-level engine alternation (not per-half):
                            # each engine owns every other pair, keeping the
                            # two exp engines decoupled by a full pair of
                            # slack (half-level splitting lockstepped them
                            # and measured slower).
                            ip = ip_counter[0]
                            ip_counter[0] += 1
                            use_dve = ip % 2 == 0

                            def emit_exp(pt, st, lo, hi, use_dve):
                                if use_dve:
                                    nc.vector.tensor_scalar(
                                        pt[:, lo:hi].bitcast(mybir.dt.int16),
                                        st[:, lo:hi],
                                        EXP_A,
                                        EXP_B,
                                        mybir.AluOpType.mult,
                                        mybir.AluOpType.add,
                                    )
                                else:
                                    nc.scalar.activation(
                                        pt[:, lo:hi],
                                        st[:, lo:hi],
                                        mybir.ActivationFunctionType.Exp,
                                        scale=SCALE,
                                    )

                            def consume(
                                st=st, pt=pt, offs=offs, kt0=kt0, qg=qg, b=b,
                                out_ps=out_ps, res_drain_bank=res_drain_bank,
                                is_diag=is_diag, emit_exp=emit_exp,
                                use_dve=use_dve,
                            ):
                                # exp split per 512-col half so PV(half0)
                                # waits only on half0 (region-level deps).
                                # Diag pairs put the even-j (self-attention)
                                # halves on ScalarE (exact exp) for accuracy.
                                if not is_diag:
                                    emit_exp(pt, st, 0, 512, use_dve)
                                    emit_exp(pt, st, 512, 1024, use_dve)
                                for half in range(2):
                                    kt_i = kt0 + half
                                    j = kt_i - 4 * qg
                                    q_off = max(0, j) * QBLK
                                    base = half * 512
                                    if j >= 0:
                                        # diag halves split across engines:
                                        # large halves (j0, j2) on the DVE,
                                        # small (j1, j3) exact on ScalarE —
                                        # keeps ScalarE's qg-end burst short
                                        # (measured faster than the flip)
                                        emit_exp(
                                            pt, st, base + q_off, base + 512,
                                            j % 2 == 0,
                                        )
                                        # zero exp where q < k in diag block
                                        nc.gpsimd.affine_select(
                                            out=pt[:, base + q_off : base + q_off + QBLK],
                                            in_=pt[:, base + q_off : base + q_off + QBLK],
                                            compare_op=mybir.AluOpType.is_ge,
                                            fill=0.0,
                                            base=0,
                                            channel_multiplier=-1,
                                            pattern=[[1, QBLK]],
                                        )
                                    for qb in range(max(0, j), 4):
                                        # only the bank's first chain issues
                                        # start=True (it clears has_written for
                                        # the WHOLE bank); the partner chain's
                                        # first write lands on cleared bits and
                                        # overwrites by the per-element rule.
                                        nc.tensor.matmul(
                                            out_ps[qb],
                                            lhsT=pt[
                                                :,
                                                base + qb * QBLK : base + (qb + 1) * QBLK,
                                            ],
                                            rhs=pv_rhs(b, kt_i),
                                            start=(kt_i == 0 and qb % 2 == 0),
                                            stop=(kt_i == 4 * qg + qb),
                                        )
                                if kt0 >= 4 * qg:
                                    res_drain_bank((kt0 - 4 * qg) // 2)

                            push_pending(consume)
            flush_pending()
    _split_waits(nc)
    return nc


def _get_nc():
    if "nc" not in _CACHE:
        _CACHE["nc"] = _build_nc()
    return _CACHE["nc"]


def _prep_inputs(query, key, value):
    """Host-side shard + layout prep: slice heads per core, transpose q/k to
    [d, s], cast to bf16."""
    bf16 = ml_dtypes.bfloat16
    q_bf = np.asarray(query, dtype=np.float32).astype(bf16)
    k_bf = np.asarray(key, dtype=np.float32).astype(bf16)
    v_bf = np.asarray(value, dtype=np.float32).astype(bf16)

    in_maps = []
    for c in range(NCORES):
        qc = q_bf[:, :, c * HPC : (c + 1) * HPC, :]  # [B, S, HPC, D]
        qT = np.ascontiguousarray(qc.transpose(0, 2, 3, 1))  # [B, HPC, D, S]
        kc = k_bf[:, :, c, :]  # [B, S, D]
        kT = np.ascontiguousarray(kc.transpose(0, 2, 1))  # [B, D, S]
        vc = v_bf[:, :, c, :]  # [B, S, D]
        # device layout [B, 4, KT, 4*(D+1)]: chunk ch holds k-tiles
        # 4ch..4ch+3; per k-tile 129 cols with the ones column FIRST
        vt = np.empty((B, 4, KT, 4, D + 1), dtype=v_bf.dtype)
        vt[..., 0] = 1.0
        vt[..., 1:] = (
            vc.reshape(B, 4, 4, KT, D)  # [b, ch, kt_local, p, d]
            .transpose(0, 1, 3, 2, 4)   # [b, ch, p, kt_local, d]
        )
        vc = np.ascontiguousarray(vt.reshape(B, 4, KT, 4 * (D + 1)))
        in_maps.append({"qT": qT, "kT": kT, "v": vc})
    return in_maps


def _assemble(results):
    outs = []
    for c in range(NCORES):
        o = results[c]["out"]  # [B, HPC, S//QBLK, QBLK, D]
        o = o.transpose(0, 2, 3, 1, 4).reshape(B, S, HPC, D)
        outs.append(o)
    return np.concatenate(outs, axis=2)  # [B, S, NH, D]


def _install_ntff_hook():
    """Recreate antenv.axon_hooks (absent in this container) so
    run_bass_kernel_spmd(trace=True) can collect NTFF profiles."""
    import sys, types

    if "antenv.axon_hooks" in sys.modules:
        return
    from trn_agent_boot.trn_boot import _ntff_profile_via_ctypes

    hook = _ntff_profile_via_ctypes("/opt/axon/libaxon_pjrt.so")
    mod = types.ModuleType("antenv.axon_hooks")
    mod.get_axon_ntff_profile_hook = lambda: hook
    sys.modules["antenv.axon_hooks"] = mod


def run(query, key, value, attn_mask=None, trace=False):
    """Run the SDPA kernel; returns (out [B,S,NH,D] f32, exec_time_ns|None)."""
    from concourse.bass_utils import run_bass_kernel_spmd

    if trace:
        _install_ntff_hook()
    nc = _get_nc()
    in_maps = _prep_inputs(query, key, value)
    res = run_bass_kernel_spmd(
        nc, in_maps, core_ids=list(range(NCORES)), trace=trace
    )
    return _assemble(res.results), res.exec_time_ns


def kernel(query, key, value, attn_mask=None):
    out, _ = run(query, key, value, attn_mask)
    return out



# revision 36
# speedup vs baseline: 1.1613x; 1.1613x over previous
"""Causal GQA SDPA on 8 Trainium2 NeuronCores (Bass/Tile).

Problem: B=2, S=2048, NH=32 query heads, NKV=8 kv heads, D=128, f32 I/O,
causal additive mask. Sharding: tensor-parallel over query heads — core c
gets q heads [4c, 4c+4) for both batches, which map exactly onto kv head c
(GQA group size 4), so k/v need no replication across cores.

Per-core kernel (all compute in bf16, f32 PSUM accumulation):
  scores^T[k, q] = K^T(stationary) x Q^T(moving)  -> PSUM [128k, 512q]
  P^T = exp(scale * scores^T)                      -> SBUF bf16
  causal: upper-triangle zeroed via gpsimd.affine_select on the diagonal
          128x128 block; fully-masked blocks are never computed.
  out[q, d+1] = P^T(stationary) x [V | 1](moving)  -> PSUM [128q, 129]
  column 0 accumulates the softmax denominator; divide via DVE
  reciprocal + per-partition multiply (split ScalarE/DVE), DMA out f32.

The exp is the throughput bottleneck (ScalarE is 1 elem/cycle/lane), so
it is SPLIT between two engines: ScalarE runs exact ACTIVATE(Exp) and the
Vector engine runs a Schraudolph-style approximation in one tensor_scalar
op  (i16 = trunc(A*s + B), bitcast to bf16 ~= exp(s*SCALE), ~1.8% rms) —
k-tile pairs alternate engines, and diagonal halves put j1/j3 on ScalarE.
PV/QK consumers trail their QK pair by two pairs (PIPE_DEPTH) so the exp
latency hides behind PE work; the 4 PV accumulators share 2 PSUM banks
(only the first chain per bank issues start=True — start clears the
whole bank's has_written bits) freeing banks for triple-buffered scores.

No max-subtraction is needed: scores ~ N(0,1) after scaling, exp is far
from overflow, and exp(score - 1e9) underflows to exactly 0.0 in f32 just
like the reference's softmax(score + mask).
"""

import math
import numpy as np
import ml_dtypes

B = 2
S = 2048
NH = 32
NKV = 8
D = 128
NCORES = 8
HPC = NH // NCORES          # q heads per core = 4
QG = 4                      # q-groups of 512 per (b, h)
QBLK = 128                  # q rows per PSUM out tile
KT = 128                    # k rows per k-tile
NKT = S // KT               # 16 k-tiles
SCALE = 1.0 / math.sqrt(D)

# Schraudolph-style exp on the DVE: i16 = trunc_f32(A*s + B) bitcast to bf16
# approximates exp(s*SCALE) (rel err ~1.8% rms, calibrated C=-7 for the
# truncating f32->i16 convert). Used to offload part of the softmax exp from
# the Scalar engine (the bottleneck) onto the otherwise-idle Vector engine.
EXP_A = float(128.0 * math.log2(math.e) * SCALE)
EXP_B = float(127 * 128 - 7.0)

_CACHE = {}


def _split_waits(nc, max_waits=1):
    """The walrus build in this container rejects instructions carrying more
    than one sync-wait ("Too many sync wait commands"). Engine queues
    dispatch in order, so excess waits can ride on NOPs inserted just before
    the instruction on the same engine — semantically identical gating."""
    import concourse.mybir as mybir

    n = 0
    for fn in nc.m.functions:
        for bb in fn.blocks:
            new = []
            changed = False
            for ins in bb.instructions:
                si = ins.sync_info
                waits = list(si.on_wait) if si is not None and si.on_wait else []
                if len(waits) > max_waits:
                    for w in waits[:-max_waits]:
                        n += 1
                        nop = mybir.InstNoOp(
                            name=f"I-waitsplit-{n}", ins=[], outs=[]
                        )
                        nop.engine = ins.engine
                        nop.sync_info = mybir.SyncInfo(on_wait=[w], on_update=[])
                        new.append(nop)
                    ins.sync_info = mybir.SyncInfo(
                        on_wait=waits[-max_waits:], on_update=list(si.on_update)
                    )
                    changed = True
                new.append(ins)
            if changed:
                bb.instructions = new


def _build_nc():
    import concourse.bass as bass
    import concourse.mybir as mybir

    f32 = mybir.dt.float32
    bf16 = mybir.dt.bfloat16

    nc = bass.Bass()
    qT = nc.declare_dram_parameter("qT", [B, HPC, D, S], bf16, isOutput=False)
    kT = nc.declare_dram_parameter("kT", [B, D, S], bf16, isOutput=False)
    v = nc.declare_dram_parameter("v", [B, 4, KT, 4 * (D + 1)], bf16, isOutput=False)
    out = nc.declare_dram_parameter(
        "out", [B, HPC, S // QBLK, QBLK, D], f32, isOutput=True
    )

    from concourse.tile import TileContext

    with TileContext(nc) as tc:
        with (
            tc.tile_pool(name="kv", bufs=1) as kv_pool,
            tc.tile_pool(name="q", bufs=8) as q_pool,
            tc.tile_pool(name="pt", bufs=8) as pt_pool,
            tc.tile_pool(name="res", bufs=12) as res_pool,
            tc.tile_pool(name="st", bufs=3, space="PSUM") as st_pool,
            tc.tile_pool(name="acc", bufs=2, space="PSUM") as acc_pool,
        ):
            # Warm the ScalarE activation table set at t~0: the implicit
            # ACT_TABLE_LOAD (~2.7us) then overlaps the initial input DMAs
            # instead of delaying the first real EXP.
            warm = res_pool.tile([128, 1], f32, tag="warm")
            nc.vector.memset(warm[:], 0.0)
            nc.scalar.activation(
                warm[:], warm[:], mybir.ActivationFunctionType.Exp
            )

            # Persistent K^T and V~ per batch, in chunk-granular tiles so the
            # first QK matmul waits on ~0.75MB of DMA, not ~3MB (Tile tracks
            # dependencies per tile).
            kt_sb = {}  # (b, ch) -> [D, 512] K^T cols for k-tiles 4ch..4ch+3
            v_sb = {}   # (b, ch) -> [KT, 4, D+1] V rows + ones col


            def load_kv_chunk(b, ch, what="kv", split=False):
                if "k" in what:
                    k_tile = kv_pool.tile(
                        [D, 512], bf16, tag=f"kt{b}c{ch}", name=f"ktile{b}{ch}"
                    )
                    if split:
                        # 4 sub-DMAs land on parallel queues so the first
                        # QK's 128-col slice arrives ~4x sooner at startup
                        for i in range(4):
                            nc.sync.dma_start(
                                k_tile[:, i * 128 : (i + 1) * 128],
                                kT[b][:, ch * 512 + i * 128 : ch * 512 + (i + 1) * 128],
                            )
                    else:
                        nc.sync.dma_start(
                            k_tile[:], kT[b][:, ch * 512 : (ch + 1) * 512]
                        )
                    kt_sb[(b, ch)] = k_tile
                if "v" in what:
                    # host layout [KT, 4*(D+1)]: per k-tile 129 cols, ones first
                    v_tile = kv_pool.tile(
                        [KT, 4, D + 1], bf16, tag=f"v{b}c{ch}", name=f"vtile{b}{ch}"
                    )
                    nc.sync.dma_start(v_tile[:], v[b, ch])
                    v_sb[(b, ch)] = v_tile

            def qk_lhsT(b, kt_i):
                return kt_sb[(b, kt_i // 4)][:, (kt_i % 4) * KT : (kt_i % 4 + 1) * KT]

            def pv_rhs(b, kt_i):
                return v_sb[(b, kt_i // 4)][:, kt_i % 4, :]

            # Global software pipeline: PV/exp consumers of pair p are
            # emitted two pairs behind its QK matmuls, so in PE program
            # order two more QK groups (plus older PVs) separate scores
            # production from probability consumption — enough slack
            # (~1.7us of PE work) to hide the ~1.2us exp latency.
            pending = []
            PIPE_DEPTH = 2

            def push_pending(fn):
                pending.append(fn)
                while len(pending) > PIPE_DEPTH:
                    pending.pop(0)()

            def flush_pending():
                while pending:
                    pending.pop(0)()

            bh_list = [(b, h) for b in range(B) for h in range(HPC)]
            q_sb_all = {}

            def load_q(b, h, split=False):
                q_sb_all[(b, h)] = {}
                for qg in range(QG):
                    q_tile = q_pool.tile(
                        [D, 512], bf16, tag=f"qg{qg}", name=f"qtile{qg}"
                    )
                    if split and qg == 0:
                        for i in range(4):
                            nc.sync.dma_start(
                                q_tile[:, i * 128 : (i + 1) * 128],
                                qT[b, h][:, i * 128 : (i + 1) * 128],
                            )
                    else:
                        nc.sync.dma_start(
                            q_tile[:], qT[b, h][:, qg * 512 : (qg + 1) * 512]
                        )
                    q_sb_all[(b, h)][qg] = q_tile
                    if h == 0:
                        # pair0 deps (q0,k0) first; v0 right after
                        load_kv_chunk(b, qg, "k", split=(split and qg == 0))
                        load_kv_chunk(b, qg, "v")

            for idx, (b, h) in enumerate(bh_list):
                    if idx == 0:
                        load_q(b, h, split=True)
                    q_sb = q_sb_all[(b, h)]
                    ip_counter = [0]

                    for qg in range(QG):
                        if qg == 2 and idx + 1 < len(bh_list):
                            # prefetch next head's inputs mid-compute so the
                            # h-boundary has no DMA-queue collision
                            load_q(*bh_list[idx + 1])
                        n_kt = 4 * qg + 4
                        # two q-block accumulators share one PSUM bank
                        # ([128, 2, 129] f32 = 1032B/partition) so all four
                        # fit in 2 banks, freeing space for st triple-buffering
                        acc_t = [
                            acc_pool.tile(
                                [QBLK, 2, D + 1], f32, tag="acc", name=f"acc{i}"
                            )
                            for i in range(2)
                        ]
                        out_ps = [acc_t[i // 2][:, i % 2, :] for i in range(4)]

                        def res_drain_bank(t, qg=qg, b=b, h=h, acc_t=acc_t):
                            # normalize + store the 2 q-blocks of acc bank t.
                            # Both its chains close with diag pair t, one pair
                            # before the qg ends for bank 0 — draining per
                            # bank unblocks the next qg's PV WAR early while
                            # PE only ever writes the OTHER bank (collision-
                            # safe). Reciprocal batched on the DVE; one
                            # multiply on ScalarE, one on the DVE (balance).
                            rec = res_pool.tile([QBLK, 2], f32, tag=f"rec{t}")
                            nc.vector.reciprocal(rec[:], acc_t[t][:, :, 0])
                            for half in range(2):
                                qb = 2 * t + half
                                osb = res_pool.tile([QBLK, D], f32, tag="osb")
                                acc_ap = acc_t[t][:, half, 1 : D + 1]
                                rec_ap = rec[:, half : half + 1]
                                if half == 1:
                                    nc.vector.tensor_scalar_mul(
                                        osb[:], acc_ap, rec_ap
                                    )
                                else:
                                    nc.scalar.activation(
                                        osb[:],
                                        acc_ap,
                                        mybir.ActivationFunctionType.Copy,
                                        bias=0.0,
                                        scale=rec_ap,
                                    )
                                nc.sync.dma_start(out[b, h, qg * 4 + qb], osb[:])

                        # k-tiles in pairs: one [128,1024] PSUM tile and one
                        # wide ACTIVATE (amortizes the 352-cycle overhead).
                        for ktp in range(n_kt // 2):
                            kt0 = 2 * ktp
                            st = st_pool.tile([KT, 1024], f32)
                            pt = pt_pool.tile([KT, 1024], bf16, tag="pt")
                            offs = []
                            for half in range(2):
                                kt_i = kt0 + half
                                j = kt_i - 4 * qg  # >= 0 on the diagonal band
                                q_off = max(0, j) * QBLK
                                offs.append(q_off)
                                nc.tensor.matmul(
                                    st[:, half * 512 + q_off : (half + 1) * 512],
                                    lhsT=qk_lhsT(b, kt_i),
                                    rhs=q_sb[qg][:, q_off:512],
                                    start=True,
                                    stop=True,
                                )

                            is_diag = kt0 + 1 >= 4 * qg
                            # pair-level engine alternation (not per-half):
                            # each engine owns every other pair, keeping the
                            # two exp engines decoupled by a full pair of
                            # slack (half-level splitting lockstepped them
                            # and measured slower).
                            ip = ip_counter[0]
                            ip_counter[0] += 1
                            use_dve = ip % 2 == 0

                            def emit_exp(pt, st, lo, hi, use_dve):
                                if use_dve:
                                    nc.vector.tensor_scalar(
                                        pt[:, lo:hi].bitcast(mybir.dt.int16),
                                        st[:, lo:hi],
                                        EXP_A,
                                        EXP_B,
                                        mybir.AluOpType.mult,
                                        mybir.AluOpType.add,
                                    )
                                else:
                                    nc.scalar.activation(
                                        pt[:, lo:hi],
                                        st[:, lo:hi],
                                        mybir.ActivationFunctionType.Exp,
                                        scale=SCALE,
                                    )

                            def consume(
                                st=st, pt=pt, offs=offs, kt0=kt0, qg=qg, b=b,
                                out_ps=out_ps, res_drain_bank=res_drain_bank,
                                is_diag=is_diag, emit_exp=emit_exp,
                                use_dve=use_dve,
                            ):
                                # exp split per 512-col half so PV(half0)
                                # waits only on half0 (region-level deps).
                                # Diag pairs put the even-j (self-attention)
                                # halves on ScalarE (exact exp) for accuracy.
                                if not is_diag:
                                    emit_exp(pt, st, 0, 512, use_dve)
                                    emit_exp(pt, st, 512, 1024, use_dve)
                                for half in range(2):
                                    kt_i = kt0 + half
                                    j = kt_i - 4 * qg
                                    q_off = max(0, j) * QBLK
                                    base = half * 512
                                    if j >= 0:
                                        # diag halves split across engines:
                                        # large halves (j0, j2) on the DVE,
                                        # small (j1, j3) exact on ScalarE —
                                        # keeps ScalarE's qg-end burst short
                                        # (measured faster than the flip)
                                        emit_exp(
                                            pt, st, base + q_off, base + 512,
                                            j % 2 == 0,
                                        )
                                        # zero exp where q < k in diag block
                                        nc.gpsimd.affine_select(
                                            out=pt[:, base + q_off : base + q_off + QBLK],
                                            in_=pt[:, base + q_off : base + q_off + QBLK],
                                            compare_op=mybir.AluOpType.is_ge,
                                            fill=0.0,
                                            base=0,
                                            channel_multiplier=-1,
                                            pattern=[[1, QBLK]],
                                        )
                                    # mask-gated PV (qb == j) goes LAST so
                                    # the PE runs the independent q-blocks
                                    # while the gpsimd select finishes
                                    qb_order = (
                                        list(range(j + 1, 4)) + [j]
                                        if j >= 0
                                        else list(range(4))
                                    )
                                    seen_banks = set()
                                    for qb in qb_order:
                                        # only the bank's FIRST chain in
                                        # emission order issues start=True
                                        # (it clears has_written for the
                                        # WHOLE bank); the partner chain's
                                        # first write lands on cleared bits
                                        # and overwrites per-element.
                                        first_in_bank = (
                                            qb // 2 not in seen_banks
                                        )
                                        seen_banks.add(qb // 2)
                                        nc.tensor.matmul(
                                            out_ps[qb],
                                            lhsT=pt[
                                                :,
                                                base + qb * QBLK : base + (qb + 1) * QBLK,
                                            ],
                                            rhs=pv_rhs(b, kt_i),
                                            start=(kt_i == 0 and first_in_bank),
                                            stop=(kt_i == 4 * qg + qb),
                                        )
                                if kt0 >= 4 * qg:
                                    res_drain_bank((kt0 - 4 * qg) // 2)

                            push_pending(consume)
            flush_pending()
    _split_waits(nc)
    return nc


def _get_nc():
    if "nc" not in _CACHE:
        _CACHE["nc"] = _build_nc()
    return _CACHE["nc"]


def _prep_inputs(query, key, value):
    """Host-side shard + layout prep: slice heads per core, transpose q/k to
    [d, s], cast to bf16."""
    bf16 = ml_dtypes.bfloat16
    q_bf = np.asarray(query, dtype=np.float32).astype(bf16)
    k_bf = np.asarray(key, dtype=np.float32).astype(bf16)
    v_bf = np.asarray(value, dtype=np.float32).astype(bf16)

    in_maps = []
    for c in range(NCORES):
        qc = q_bf[:, :, c * HPC : (c + 1) * HPC, :]  # [B, S, HPC, D]
        qT = np.ascontiguousarray(qc.transpose(0, 2, 3, 1))  # [B, HPC, D, S]
        kc = k_bf[:, :, c, :]  # [B, S, D]
        kT = np.ascontiguousarray(kc.transpose(0, 2, 1))  # [B, D, S]
        vc = v_bf[:, :, c, :]  # [B, S, D]
        # device layout [B, 4, KT, 4*(D+1)]: chunk ch holds k-tiles
        # 4ch..4ch+3; per k-tile 129 cols with the ones column FIRST
        vt = np.empty((B, 4, KT, 4, D + 1), dtype=v_bf.dtype)
        vt[..., 0] = 1.0
        vt[..., 1:] = (
            vc.reshape(B, 4, 4, KT, D)  # [b, ch, kt_local, p, d]
            .transpose(0, 1, 3, 2, 4)   # [b, ch, p, kt_local, d]
        )
        vc = np.ascontiguousarray(vt.reshape(B, 4, KT, 4 * (D + 1)))
        in_maps.append({"qT": qT, "kT": kT, "v": vc})
    return in_maps


def _assemble(results):
    outs = []
    for c in range(NCORES):
        o = results[c]["out"]  # [B, HPC, S//QBLK, QBLK, D]
        o = o.transpose(0, 2, 3, 1, 4).reshape(B, S, HPC, D)
        outs.append(o)
    return np.concatenate(outs, axis=2)  # [B, S, NH, D]


def _install_ntff_hook():
    """Recreate antenv.axon_hooks (absent in this container) so
    run_bass_kernel_spmd(trace=True) can collect NTFF profiles."""
    import sys, types

    if "antenv.axon_hooks" in sys.modules:
        return
    from trn_agent_boot.trn_boot import _ntff_profile_via_ctypes

    hook = _ntff_profile_via_ctypes("/opt/axon/libaxon_pjrt.so")
    mod = types.ModuleType("antenv.axon_hooks")
    mod.get_axon_ntff_profile_hook = lambda: hook
    sys.modules["antenv.axon_hooks"] = mod


def run(query, key, value, attn_mask=None, trace=False):
    """Run the SDPA kernel; returns (out [B,S,NH,D] f32, exec_time_ns|None)."""
    from concourse.bass_utils import run_bass_kernel_spmd

    if trace:
        _install_ntff_hook()
    nc = _get_nc()
    in_maps = _prep_inputs(query, key, value)
    res = run_bass_kernel_spmd(
        nc, in_maps, core_ids=list(range(NCORES)), trace=trace
    )
    return _assemble(res.results), res.exec_time_ns


def kernel(query, key, value, attn_mask=None):
    out, _ = run(query, key, value, attn_mask)
    return out



# revision 40
# speedup vs baseline: 1.2077x; 1.0400x over previous
"""Causal GQA SDPA on 8 Trainium2 NeuronCores (Bass/Tile).

Problem: B=2, S=2048, NH=32 query heads, NKV=8 kv heads, D=128, f32 I/O,
causal additive mask. Sharding: tensor-parallel over query heads — core c
gets q heads [4c, 4c+4) for both batches, which map exactly onto kv head c
(GQA group size 4), so k/v need no replication across cores.

Per-core kernel (all compute in bf16, f32 PSUM accumulation):
  scores^T[k, q] = K^T(stationary) x Q^T(moving)  -> PSUM [128k, 512q]
  P^T = exp(scale * scores^T)                      -> SBUF bf16
  causal: upper-triangle zeroed via gpsimd.affine_select on the diagonal
          128x128 block; fully-masked blocks are never computed.
  out[q, d+1] = P^T(stationary) x [V | 1](moving)  -> PSUM [128q, 129]
  column 0 accumulates the softmax denominator; divide via DVE
  reciprocal + per-partition multiply (split ScalarE/DVE), DMA out f32.

The exp is the throughput bottleneck (ScalarE is 1 elem/cycle/lane), so
it is SPLIT between two engines: ScalarE runs exact ACTIVATE(Exp) and the
Vector engine runs a Schraudolph-style approximation in one tensor_scalar
op  (i16 = trunc(A*s + B), bitcast to bf16 ~= exp(s*SCALE), ~1.8% rms) —
k-tile pairs alternate engines, and diagonal halves put j1/j3 on ScalarE.
PV/QK consumers trail their QK pair by two pairs (PIPE_DEPTH) so the exp
latency hides behind PE work; the 4 PV accumulators share 2 PSUM banks
(only the first chain per bank issues start=True — start clears the
whole bank's has_written bits) freeing banks for triple-buffered scores.

No max-subtraction is needed: scores ~ N(0,1) after scaling, exp is far
from overflow, and exp(score - 1e9) underflows to exactly 0.0 in f32 just
like the reference's softmax(score + mask).
"""

import math
import numpy as np
import ml_dtypes

B = 2
S = 2048
NH = 32
NKV = 8
D = 128
NCORES = 8
HPC = NH // NCORES          # q heads per core = 4
QG = 4                      # q-groups of 512 per (b, h)
QBLK = 128                  # q rows per PSUM out tile
KT = 128                    # k rows per k-tile
NKT = S // KT               # 16 k-tiles
SCALE = 1.0 / math.sqrt(D)

# Schraudolph-style exp on the DVE: i16 = trunc_f32(A*s + B) bitcast to bf16
# approximates exp(s*SCALE) (rel err ~1.8% rms, calibrated C=-7 for the
# truncating f32->i16 convert). Used to offload part of the softmax exp from
# the Scalar engine (the bottleneck) onto the otherwise-idle Vector engine.
EXP_A = float(128.0 * math.log2(math.e) * SCALE)
EXP_B = float(127 * 128 - 7.0)

_CACHE = {}


def _split_waits(nc, max_waits=1):
    """The walrus build in this container rejects instructions carrying more
    than one sync-wait ("Too many sync wait commands"). Engine queues
    dispatch in order, so excess waits can ride on NOPs inserted just before
    the instruction on the same engine — semantically identical gating."""
    import concourse.mybir as mybir

    n = 0
    for fn in nc.m.functions:
        for bb in fn.blocks:
            new = []
            changed = False
            for ins in bb.instructions:
                si = ins.sync_info
                waits = list(si.on_wait) if si is not None and si.on_wait else []
                if len(waits) > max_waits:
                    for w in waits[:-max_waits]:
                        n += 1
                        nop = mybir.InstNoOp(
                            name=f"I-waitsplit-{n}", ins=[], outs=[]
                        )
                        nop.engine = ins.engine
                        nop.sync_info = mybir.SyncInfo(on_wait=[w], on_update=[])
                        new.append(nop)
                    ins.sync_info = mybir.SyncInfo(
                        on_wait=waits[-max_waits:], on_update=list(si.on_update)
                    )
                    changed = True
                new.append(ins)
            if changed:
                bb.instructions = new


def _build_nc():
    import concourse.bass as bass
    import concourse.mybir as mybir

    f32 = mybir.dt.float32
    bf16 = mybir.dt.bfloat16

    nc = bass.Bass()
    qT = nc.declare_dram_parameter("qT", [B, HPC, D, S], bf16, isOutput=False)
    kT = nc.declare_dram_parameter("kT", [B, D, S], bf16, isOutput=False)
    v = nc.declare_dram_parameter("v", [B, 4, KT, 4 * (D + 1)], bf16, isOutput=False)
    out = nc.declare_dram_parameter(
        "out", [B, HPC, S // QBLK, QBLK, D], f32, isOutput=True
    )

    from concourse.tile import TileContext

    with TileContext(nc) as tc:
        with (
            tc.tile_pool(name="kv", bufs=1) as kv_pool,
            tc.tile_pool(name="q", bufs=8) as q_pool,
            tc.tile_pool(name="pt", bufs=8) as pt_pool,
            tc.tile_pool(name="res", bufs=12) as res_pool,
            tc.tile_pool(name="st", bufs=3, space="PSUM") as st_pool,
            tc.tile_pool(name="acc", bufs=2, space="PSUM") as acc_pool,
        ):
            # Warm the ScalarE activation table set at t~0: the implicit
            # ACT_TABLE_LOAD (~2.7us) then overlaps the initial input DMAs
            # instead of delaying the first real EXP.
            warm = res_pool.tile([128, 1], f32, tag="warm")
            nc.vector.memset(warm[:], 0.0)
            nc.scalar.activation(
                warm[:], warm[:], mybir.ActivationFunctionType.Exp
            )

            # Persistent K^T and V~ per batch, in chunk-granular tiles so the
            # first QK matmul waits on ~0.75MB of DMA, not ~3MB (Tile tracks
            # dependencies per tile).
            kt_sb = {}  # (b, ch) -> [D, 512] K^T cols for k-tiles 4ch..4ch+3
            v_sb = {}   # (b, ch) -> [KT, 4, D+1] V rows + ones col


            def load_kv_chunk(b, ch, what="kv", split=False):
                if "k" in what and (b, ch) in kt_sb:
                    what = what.replace("k", "")
                if "v" in what and (b, ch) in v_sb:
                    what = what.replace("v", "")
                if "k" in what:
                    k_tile = kv_pool.tile(
                        [D, 512], bf16, tag=f"kt{b}c{ch}", name=f"ktile{b}{ch}"
                    )
                    if split:
                        # 4 sub-DMAs land on parallel queues so the first
                        # QK's 128-col slice arrives ~4x sooner at startup
                        for i in range(4):
                            nc.sync.dma_start(
                                k_tile[:, i * 128 : (i + 1) * 128],
                                kT[b][:, ch * 512 + i * 128 : ch * 512 + (i + 1) * 128],
                            )
                    else:
                        nc.sync.dma_start(
                            k_tile[:], kT[b][:, ch * 512 : (ch + 1) * 512]
                        )
                    kt_sb[(b, ch)] = k_tile
                if "v" in what:
                    # host layout [KT, 4*(D+1)]: per k-tile 129 cols, ones first
                    v_tile = kv_pool.tile(
                        [KT, 4, D + 1], bf16, tag=f"v{b}c{ch}", name=f"vtile{b}{ch}"
                    )
                    nc.sync.dma_start(v_tile[:], v[b, ch])
                    v_sb[(b, ch)] = v_tile

            def qk_lhsT(b, kt_i):
                return kt_sb[(b, kt_i // 4)][:, (kt_i % 4) * KT : (kt_i % 4 + 1) * KT]

            def pv_rhs(b, kt_i):
                return v_sb[(b, kt_i // 4)][:, kt_i % 4, :]

            # Global software pipeline: PV/exp consumers of pair p are
            # emitted two pairs behind its QK matmuls, so in PE program
            # order two more QK groups (plus older PVs) separate scores
            # production from probability consumption — enough slack
            # (~1.7us of PE work) to hide the ~1.2us exp latency.
            pending = []
            PIPE_DEPTH = 2

            def push_pending(fn):
                pending.append(fn)
                while len(pending) > PIPE_DEPTH:
                    pending.pop(0)()

            def flush_pending():
                while pending:
                    pending.pop(0)()

            bh_list = [(b, h) for b in range(B) for h in range(HPC)]
            q_sb_all = {}

            def load_q(b, h, split=False):
                q_sb_all[(b, h)] = {}
                for qg in range(QG):
                    q_tile = q_pool.tile(
                        [D, 512], bf16, tag=f"qg{qg}", name=f"qtile{qg}"
                    )
                    if split and qg == 0:
                        for i in range(4):
                            nc.sync.dma_start(
                                q_tile[:, i * 128 : (i + 1) * 128],
                                qT[b, h][:, i * 128 : (i + 1) * 128],
                            )
                    else:
                        nc.sync.dma_start(
                            q_tile[:], qT[b, h][:, qg * 512 : (qg + 1) * 512]
                        )
                    q_sb_all[(b, h)][qg] = q_tile
                    if h == 0:
                        # pair0 deps (q0,k0) first; v0 right after
                        load_kv_chunk(b, qg, "k", split=(split and qg == 0))
                        load_kv_chunk(b, qg, "v")

            for idx, (b, h) in enumerate(bh_list):
                    if idx == 0:
                        load_q(b, h)
                    q_sb = q_sb_all[(b, h)]
                    ip_counter = [0]

                    for qg in range(QG):
                        if idx == 1 and qg == 1:
                            # prefetch batch 1's K/V early, while the DMA
                            # queues are quiet — loading them at the batch
                            # boundary cost a multi-us PE bubble
                            for ch in range(4):
                                load_kv_chunk(1, ch, "kv")
                        if qg == 2 and idx + 1 < len(bh_list):
                            # prefetch next head's inputs mid-compute so the
                            # h-boundary has no DMA-queue collision
                            load_q(*bh_list[idx + 1])
                        n_kt = 4 * qg + 4
                        # two q-block accumulators share one PSUM bank
                        # ([128, 2, 129] f32 = 1032B/partition) so all four
                        # fit in 2 banks, freeing space for st triple-buffering
                        acc_t = [
                            acc_pool.tile(
                                [QBLK, 2, D + 1], f32, tag="acc", name=f"acc{i}"
                            )
                            for i in range(2)
                        ]
                        out_ps = [acc_t[i // 2][:, i % 2, :] for i in range(4)]

                        def res_drain_bank(t, qg=qg, b=b, h=h, acc_t=acc_t):
                            # normalize + store the 2 q-blocks of acc bank t.
                            # Both its chains close with diag pair t, one pair
                            # before the qg ends for bank 0 — draining per
                            # bank unblocks the next qg's PV WAR early while
                            # PE only ever writes the OTHER bank (collision-
                            # safe). Reciprocal batched on the DVE; one
                            # multiply on ScalarE, one on the DVE (balance).
                            rec = res_pool.tile([QBLK, 2], f32, tag=f"rec{t}")
                            nc.vector.reciprocal(rec[:], acc_t[t][:, :, 0])
                            for half in range(2):
                                qb = 2 * t + half
                                osb = res_pool.tile([QBLK, D], f32, tag="osb")
                                acc_ap = acc_t[t][:, half, 1 : D + 1]
                                rec_ap = rec[:, half : half + 1]
                                if half == 1:
                                    nc.vector.tensor_scalar_mul(
                                        osb[:], acc_ap, rec_ap
                                    )
                                else:
                                    nc.scalar.activation(
                                        osb[:],
                                        acc_ap,
                                        mybir.ActivationFunctionType.Copy,
                                        bias=0.0,
                                        scale=rec_ap,
                                    )
                                nc.sync.dma_start(out[b, h, qg * 4 + qb], osb[:])

                        # k-tiles in pairs: one [128,1024] PSUM tile and one
                        # wide ACTIVATE (amortizes the 352-cycle overhead).
                        for ktp in range(n_kt // 2):
                            kt0 = 2 * ktp
                            st = st_pool.tile([KT, 1024], f32)
                            pt = pt_pool.tile([KT, 1024], bf16, tag="pt")
                            offs = []
                            for half in range(2):
                                kt_i = kt0 + half
                                j = kt_i - 4 * qg  # >= 0 on the diagonal band
                                q_off = max(0, j) * QBLK
                                offs.append(q_off)
                                nc.tensor.matmul(
                                    st[:, half * 512 + q_off : (half + 1) * 512],
                                    lhsT=qk_lhsT(b, kt_i),
                                    rhs=q_sb[qg][:, q_off:512],
                                    start=True,
                                    stop=True,
                                )

                            is_diag = kt0 + 1 >= 4 * qg
                            # pair-level engine alternation (not per-half):
                            # each engine owns every other pair, keeping the
                            # two exp engines decoupled by a full pair of
                            # slack (half-level splitting lockstepped them
                            # and measured slower).
                            ip = ip_counter[0]
                            ip_counter[0] += 1
                            use_dve = ip % 2 == 0

                            def emit_exp(pt, st, lo, hi, use_dve):
                                if use_dve:
                                    nc.vector.tensor_scalar(
                                        pt[:, lo:hi].bitcast(mybir.dt.int16),
                                        st[:, lo:hi],
                                        EXP_A,
                                        EXP_B,
                                        mybir.AluOpType.mult,
                                        mybir.AluOpType.add,
                                    )
                                else:
                                    nc.scalar.activation(
                                        pt[:, lo:hi],
                                        st[:, lo:hi],
                                        mybir.ActivationFunctionType.Exp,
                                        scale=SCALE,
                                    )

                            def consume(
                                st=st, pt=pt, offs=offs, kt0=kt0, qg=qg, b=b,
                                out_ps=out_ps, res_drain_bank=res_drain_bank,
                                is_diag=is_diag, emit_exp=emit_exp,
                                use_dve=use_dve,
                            ):
                                # exp split per 512-col half so PV(half0)
                                # waits only on half0 (region-level deps).
                                # Diag pairs put the even-j (self-attention)
                                # halves on ScalarE (exact exp) for accuracy.
                                if not is_diag:
                                    emit_exp(pt, st, 0, 512, use_dve)
                                    emit_exp(pt, st, 512, 1024, use_dve)
                                for half in range(2):
                                    kt_i = kt0 + half
                                    j = kt_i - 4 * qg
                                    q_off = max(0, j) * QBLK
                                    base = half * 512
                                    if j >= 0:
                                        # diag halves split across engines:
                                        # large halves (j0, j2) on the DVE,
                                        # small (j1, j3) exact on ScalarE —
                                        # keeps ScalarE's qg-end burst short
                                        # (measured faster than the flip)
                                        emit_exp(
                                            pt, st, base + q_off, base + 512,
                                            j % 2 == 0,
                                        )
                                        # zero exp where q < k in diag block
                                        nc.gpsimd.affine_select(
                                            out=pt[:, base + q_off : base + q_off + QBLK],
                                            in_=pt[:, base + q_off : base + q_off + QBLK],
                                            compare_op=mybir.AluOpType.is_ge,
                                            fill=0.0,
                                            base=0,
                                            channel_multiplier=-1,
                                            pattern=[[1, QBLK]],
                                        )
                                    for qb in range(max(0, j), 4):
                                        # only the bank's first chain issues
                                        # start=True (it clears has_written
                                        # for the WHOLE bank); the partner
                                        # chain's first write lands on
                                        # cleared bits and overwrites
                                        # per-element.
                                        nc.tensor.matmul(
                                            out_ps[qb],
                                            lhsT=pt[
                                                :,
                                                base + qb * QBLK : base + (qb + 1) * QBLK,
                                            ],
                                            rhs=pv_rhs(b, kt_i),
                                            start=(kt_i == 0 and qb % 2 == 0),
                                            stop=(kt_i == 4 * qg + qb),
                                        )
                                if kt0 >= 4 * qg:
                                    res_drain_bank((kt0 - 4 * qg) // 2)

                            push_pending(consume)
            flush_pending()
    _split_waits(nc)
    return nc


def _get_nc():
    if "nc" not in _CACHE:
        _CACHE["nc"] = _build_nc()
    return _CACHE["nc"]


def _prep_inputs(query, key, value):
    """Host-side shard + layout prep: slice heads per core, transpose q/k to
    [d, s], cast to bf16."""
    bf16 = ml_dtypes.bfloat16
    q_bf = np.asarray(query, dtype=np.float32).astype(bf16)
    k_bf = np.asarray(key, dtype=np.float32).astype(bf16)
    v_bf = np.asarray(value, dtype=np.float32).astype(bf16)

    in_maps = []
    for c in range(NCORES):
        qc = q_bf[:, :, c * HPC : (c + 1) * HPC, :]  # [B, S, HPC, D]
        qT = np.ascontiguousarray(qc.transpose(0, 2, 3, 1))  # [B, HPC, D, S]
        kc = k_bf[:, :, c, :]  # [B, S, D]
        kT = np.ascontiguousarray(kc.transpose(0, 2, 1))  # [B, D, S]
        vc = v_bf[:, :, c, :]  # [B, S, D]
        # device layout [B, 4, KT, 4*(D+1)]: chunk ch holds k-tiles
        # 4ch..4ch+3; per k-tile 129 cols with the ones column FIRST
        vt = np.empty((B, 4, KT, 4, D + 1), dtype=v_bf.dtype)
        vt[..., 0] = 1.0
        vt[..., 1:] = (
            vc.reshape(B, 4, 4, KT, D)  # [b, ch, kt_local, p, d]
            .transpose(0, 1, 3, 2, 4)   # [b, ch, p, kt_local, d]
        )
        vc = np.ascontiguousarray(vt.reshape(B, 4, KT, 4 * (D + 1)))
        in_maps.append({"qT": qT, "kT": kT, "v": vc})
    return in_maps


def _assemble(results):
    outs = []
    for c in range(NCORES):
        o = results[c]["out"]  # [B, HPC, S//QBLK, QBLK, D]
        o = o.transpose(0, 2, 3, 1, 4).reshape(B, S, HPC, D)
        outs.append(o)
    return np.concatenate(outs, axis=2)  # [B, S, NH, D]


def _install_ntff_hook():
    """Recreate antenv.axon_hooks (absent in this container) so
    run_bass_kernel_spmd(trace=True) can collect NTFF profiles."""
    import sys, types

    if "antenv.axon_hooks" in sys.modules:
        return
    from trn_agent_boot.trn_boot import _ntff_profile_via_ctypes

    hook = _ntff_profile_via_ctypes("/opt/axon/libaxon_pjrt.so")
    mod = types.ModuleType("antenv.axon_hooks")
    mod.get_axon_ntff_profile_hook = lambda: hook
    sys.modules["antenv.axon_hooks"] = mod


def run(query, key, value, attn_mask=None, trace=False):
    """Run the SDPA kernel; returns (out [B,S,NH,D] f32, exec_time_ns|None)."""
    from concourse.bass_utils import run_bass_kernel_spmd

    if trace:
        _install_ntff_hook()
    nc = _get_nc()
    in_maps = _prep_inputs(query, key, value)
    res = run_bass_kernel_spmd(
        nc, in_maps, core_ids=list(range(NCORES)), trace=trace
    )
    return _assemble(res.results), res.exec_time_ns


def kernel(query, key, value, attn_mask=None):
    out, _ = run(query, key, value, attn_mask)
    return out

